# revision 40
# baseline (speedup 1.0000x reference)
"""Trainium2 Bass kernel for nn_Encoder_20942260535802.

Pipeline (per reference.py):
    visit_emb = H.T @ E                      # [8192, 128]
    hidden    = GRU(visit_emb)               # sequential scan, 8192 steps
    out       = softmax(hidden@attn_w) @ hidden

Strategy:
  * Shard the 8192-visit sequence across 8 cores (1024 visits each). Each
    core loads only its H columns (memory-bound phase, ~20 MB/core) and
    computes visit_embT = E.T @ Hc plus the input-gate projections locally.
  * The GRU recurrence is solved by Jacobi (fixed-point) iteration over the
    whole local chunk: h_all <- GRUStep(shift(h_all), x). The GRU map is a
    contraction (L ~ 0.86/step empirically), so K sweeps give uniform error
    ~L^K; each sweep is batched GEMM + elementwise work with no per-step
    synchronization. A 64-step warmup halo (states start from h=0 at
    chunk_start-64) makes chunks fully independent: the boundary error
    decays as L^64 ~ 7e-5 (well under the sweep error), so no inter-core
    carry is needed at all.
  * Core 0 has no real predecessor: its halo's z-gate input gets +ZBOOST so
    z=1 and the state stays exactly 0 through the halo.
  * Sweeps are column-chunked (384/384/320) so consecutive sweeps pipeline
    as a wavefront across engines; PSUM holds r/z/n per chunk in 3 banks, with
    the gate inputs pre-seeded by identity matmuls so sigma reads a single
    accumulated PSUM region. Matmuls run in float32r (single-pass PE fp32,
    ~1.5e-4 rel err measured on HW); H/E are loaded as fp16 (halves the
    memory-bound phase; validated no accuracy impact). Phase 1 runs per
    column-chunk, spreads DMA issue across the SP/ACT/GPSIMD queues, and
    shares the PSUM pool with the sweeps so early sweeps overlap the
    remaining H DMA.
  * Attention pooling is computed per-core as partials (sum_t e_t*h_t [128]
    and per-partition partials of sum_t e_t); the final 8-way sum + divide
    is host-side unsharding (tiny).
  * Measured end-to-end vs float64 reference (K=38, f32r, fp16 loads,
    WARM=64): max rel err 6.6e-4. CoreSim cost-model exec estimate: ~214 us
    (ACT-bound: sigma+tanh sweep work at ~85% occupancy; per-sweep latency
    equals the ACT busy floor).
"""

import numpy as np

VOCAB, EDIM, HDIM = 4880, 128, 128
NSEQ = 8192
NCORES = 8
WARM = 64                  # warmup halo columns (err ~0.86^64 ~ 7e-5)
CHUNK = NSEQ // NCORES     # 1024
NCOL = CHUNK + WARM        # 1088 columns per core
KSWEEPS = 40               # Jacobi sweeps (err ~ 0.86^K)
ZBOOST = 60.0              # sigmoid(x+60) == 1.0 in fp32

_cache = {}


def _bank_chunks(off, length, bank=512):
    """Split [off, off+length) at `bank`-aligned boundaries (PSUM banks)."""
    res, c, end = [], off, off + length
    while c < end:
        w = min(bank - (c % bank), end - c)
        res.append((c, w))
        c += w
    return res


def _build():
    import concourse.mybir as mybir
    from concourse import bacc
    from concourse.tile import TileContext

    f32 = mybir.dt.float32
    f32r = mybir.dt.float32r  # single-pass PE fp32 (4x faster for N>=256)
    f16 = mybir.dt.float16    # halves the H/E DMA traffic; negligible error
    AF = mybir.ActivationFunctionType
    ALU = mybir.AluOpType

    nc = bacc.Bacc(None, target_bir_lowering=False, debug=False)

    Hc = nc.dram_tensor("Hc", [VOCAB, NCOL], f16, kind="ExternalInput")
    Ein = nc.dram_tensor("Ein", [VOCAB, EDIM], f16, kind="ExternalInput")
    WihT = nc.dram_tensor("WihT", [EDIM, 3 * HDIM], f32, kind="ExternalInput")
    WhhT = nc.dram_tensor("WhhT", [HDIM, 3 * HDIM], f32r, kind="ExternalInput")
    # consts cols: 0=b_r, 1=b_z (b_ih+b_hh), 2=b_ih_n, 3=b_hh_n, 4=attn_w, 5=zboost
    consts = nc.dram_tensor("consts", [128, 8], f32, kind="ExternalInput")
    ident = nc.dram_tensor("ident", [128, 128], f32r, kind="ExternalInput")
    partial = nc.dram_tensor("partial", [128, 2], f32, kind="ExternalOutput")

    KTILES = [(k * 128, min(128, VOCAB - k * 128)) for k in range((VOCAB + 127) // 128)]
    r_chunks = _bank_chunks(0, NCOL)            # r-gate span of ps_rz
    z_chunks = _bank_chunks(NCOL, NCOL)         # z-gate span of ps_rz
    n_chunks = _bank_chunks(0, NCOL)            # ps_n

    with TileContext(nc) as tc:
        with (
            tc.tile_pool(name="fixed", bufs=1) as fixed,
            tc.tile_pool(name="state", bufs=1) as state,
        ):
            consts_sb = fixed.tile([128, 8], f32)
            nc.sync.dma_start(out=consts_sb, in_=consts[:, :])
            ident_sb = fixed.tile([128, 128], f32r)
            nc.sync.dma_start(out=ident_sb, in_=ident[:, :])
            wih_f = fixed.tile([128, 3 * HDIM], f32)
            nc.sync.dma_start(out=wih_f, in_=WihT[:, :])
            wih_sb = fixed.tile([128, 3 * HDIM], f32r)
            nc.vector.tensor_copy(wih_sb, wih_f)
            whh_sb = fixed.tile([128, 3 * HDIM], f32r)
            nc.sync.dma_start(out=whh_sb, in_=WhhT[:, :])

            ve = state.tile([128, NCOL], f32r)           # visit_embT
            xpb_rz = state.tile([128, 2 * NCOL], f32r)   # x-proj + bias, r||z
            xnb = state.tile([128, NCOL], f32)          # x-proj + b_ih_n
            hA = state.tile([128, NCOL + 1], f32r)       # col 0 = left boundary (0)
            hB = state.tile([128, NCOL + 1], f32r)

            # ---------- Phases 1+2 interleaved ----------
            # Phase 1 runs per column-chunk (E.T @ Hc[:, chunk] -> gate
            # projections for that chunk), sharing one PSUM pool with the
            # sweeps so the first sweeps overlap the remaining H loads.
            nc.gpsimd.memset(hA.bitcast(f32), 0.0)
            nc.gpsimd.memset(hB.bitcast(f32), 0.0)
            CWS = [384, 384, 320]          # per-chunk widths (sum = NCOL)
            COFF = [0, 384, 768]
            NCH = len(CWS)
            assert sum(CWS) == NCOL
            NKT = len(KTILES)
            NKF = sum(1 for _, kp in KTILES if kp == 128)  # full 128-row tiles
            Eall = state.tile([128, NKT, EDIM], f16)
            nc.sync.dma_start(
                out=Eall[:, 0:NKF, :],
                in_=Ein[0 : NKF * 128, :].rearrange("(a p) d -> p a d", p=128),
            )
            nc.sync.dma_start(
                out=Eall[: VOCAB - NKF * 128, NKF, :], in_=Ein[NKF * 128 :, :]
            )
            GRP = 13
            kgroups = [KTILES[g : g + GRP] for g in range(0, NKT, GRP)]
            # spread H-load issue across engine queues: the SP sequencer's
            # per-dma_start cost (~2.4us) serializes phase 1 otherwise
            dma_engines = [nc.sync, nc.scalar, nc.gpsimd]
            dma_i = 0
            with (
                tc.tile_pool(name="p1ld", bufs=4) as p1ld,
                tc.tile_pool(name="sw", bufs=6) as sw,
                tc.tile_pool(name="pipeps", bufs=1, space="PSUM") as pps,
            ):
                for c in range(NCH):
                    cc0, CW = COFF[c], CWS[c]
                    ve_ps = pps.tile([128, 384], f32, tag="veps", name="ve_ps")[:, 0:CW]
                    for grp in kgroups:
                        ng = len(grp)
                        nfull = sum(1 for _, kp in grp if kp == 128)
                        k0 = grp[0][0]
                        h_t = p1ld.tile([128, GRP, CW], f16, tag="h")
                        eng = dma_engines[dma_i % len(dma_engines)]
                        dma_i += 1
                        if nfull:
                            eng.dma_start(
                                out=h_t[:, 0:nfull, :],
                                in_=Hc[k0 : k0 + 128 * nfull, cc0 : cc0 + CW].rearrange(
                                    "(a p) n -> p a n", p=128
                                ),
                            )
                        for a in range(nfull, ng):
                            ka, kp = grp[a]
                            eng.dma_start(
                                out=h_t[:kp, a, :], in_=Hc[ka : ka + kp, cc0 : cc0 + CW]
                            )
                        for a, (ka, kp) in enumerate(grp):
                            ki = ka // 128
                            nc.tensor.matmul(
                                ve_ps,
                                lhsT=Eall[:kp, ki, :],
                                rhs=h_t[:kp, a, :],
                                start=(ki == 0),
                                stop=(ki == NKT - 1),
                            )
                    nc.vector.tensor_copy(ve[:, cc0 : cc0 + CW], ve_ps)
                    for g in range(3):
                        gp = pps.tile([128, CW], f32, tag="gp")
                        nc.tensor.matmul(
                            gp,
                            lhsT=wih_sb[:, g * 128 : (g + 1) * 128],
                            rhs=ve[:, cc0 : cc0 + CW],
                            start=True,
                            stop=True,
                        )
                        if g == 0:
                            nc.vector.tensor_scalar_add(
                                xpb_rz[:, cc0 : cc0 + CW], gp, consts_sb[:, 0:1]
                            )
                        elif g == 1:
                            if c == 0:
                                # warmup cols: +zboost (core 0 only; 0 elsewhere)
                                nc.vector.tensor_scalar(
                                    xpb_rz[:, NCOL : NCOL + WARM],
                                    gp[:, 0:WARM],
                                    consts_sb[:, 1:2],
                                    consts_sb[:, 5:6],
                                    ALU.add,
                                    ALU.add,
                                )
                                nc.vector.tensor_scalar_add(
                                    xpb_rz[:, NCOL + WARM : NCOL + CW],
                                    gp[:, WARM:],
                                    consts_sb[:, 1:2],
                                )
                            else:
                                nc.vector.tensor_scalar_add(
                                    xpb_rz[:, NCOL + cc0 : NCOL + cc0 + CW],
                                    gp,
                                    consts_sb[:, 1:2],
                                )
                        else:
                            nc.vector.tensor_scalar_add(
                                xnb[:, cc0 : cc0 + CW], gp, consts_sb[:, 2:3]
                            )

                # ---------- Phase 2: Jacobi sweeps (wavefront) ----------
                for k in range(KSWEEPS):
                    hprev_t, hnext_t = (hA, hB) if k % 2 == 0 else (hB, hA)
                    for c in range(NCH):
                        c0, CW = COFF[c], CWS[c]
                        hp = hprev_t[:, c0 : c0 + CW]
                        # psum layout: r at [0:CW] (bank 0), z at [512:512+CW]
                        # (bank 1), n at [1024:1024+CW] (bank 2)
                        ps = pps.tile([128, 1536], f32, tag="ps", bufs=2, name="ps")
                        nc.tensor.matmul(
                            ps[:, 0:CW],
                            lhsT=ident_sb,
                            rhs=xpb_rz[:, c0 : c0 + CW],
                            start=True,
                            stop=False,
                        )
                        nc.tensor.matmul(
                            ps[:, 512 : 512 + CW],
                            lhsT=ident_sb,
                            rhs=xpb_rz[:, NCOL + c0 : NCOL + c0 + CW],
                            start=True,
                            stop=False,
                        )
                        nc.tensor.matmul(
                            ps[:, 0:CW],
                            lhsT=whh_sb[:, 0:128],
                            rhs=hp,
                            start=False,
                            stop=True,
                        )
                        nc.tensor.matmul(
                            ps[:, 512 : 512 + CW],
                            lhsT=whh_sb[:, 128:256],
                            rhs=hp,
                            start=False,
                            stop=True,
                        )
                        nc.tensor.matmul(
                            ps[:, 1024 : 1024 + CW],
                            lhsT=whh_sb[:, 256:384],
                            rhs=hp,
                            start=True,
                            stop=True,
                        )
                        # sigmoid over r and z via a strided view (skips pads)
                        rz = sw.tile([128, 2, CW], f32, tag="rz")
                        ps_rz_view = ps.rearrange("p (b y) -> p b y", y=512)[
                            :, 0:2, 0:CW
                        ]
                        nc.scalar.activation(rz, ps_rz_view, AF.Sigmoid)
                        r = rz[:, 0, :]
                        z = rz[:, 1, :]
                        m = sw.tile([128, CW], f32, tag="m")
                        # m = (ghn + b_hh_n) * r
                        nc.vector.scalar_tensor_tensor(
                            m,
                            ps[:, 1024 : 1024 + CW],
                            consts_sb[:, 3:4],
                            r,
                            ALU.add,
                            ALU.mult,
                        )
                        t3 = sw.tile([128, CW], f32, tag="t3")
                        nc.gpsimd.tensor_add(t3, m, xnb[:, c0 : c0 + CW])
                        nt = sw.tile([128, CW], f32, tag="nt")
                        nc.scalar.activation(nt, t3, AF.Tanh)
                        # h' = (1-z)*n + z*hprev; zb and u run parallel to
                        # the tanh path so only 2 ops trail the tanh
                        zb = sw.tile([128, CW], f32, tag="zb")
                        nc.gpsimd.tensor_scalar(
                            zb, z, -1.0, 1.0, ALU.mult, ALU.add
                        )
                        u = sw.tile([128, CW], f32, tag="u")
                        nc.vector.tensor_mul(u, z, hp.bitcast(f32))
                        m3 = sw.tile([128, CW], f32, tag="m3")
                        nc.gpsimd.tensor_mul(m3, nt, zb)
                        nc.gpsimd.tensor_add(
                            hnext_t[:, 1 + c0 : 1 + c0 + CW], m3, u
                        )

            # ---------- Phase 3: attention pooling (partial) ----------
            hfin = (hA if KSWEEPS % 2 == 0 else hB)[:, 1 + WARM : 1 + NCOL]
            with (
                tc.tile_pool(name="p3", bufs=1) as p3,
                tc.tile_pool(name="p3d", bufs=2) as p3d,
                tc.tile_pool(name="p3ps", bufs=1, space="PSUM") as p3ps,
                tc.tile_pool(name="p3tp", bufs=2, space="PSUM") as p3tp,
            ):
                nj = CHUNK // 128
                s_ps = p3ps.tile([128, nj], f32)
                for j in range(nj):
                    nc.tensor.matmul(
                        s_ps[:, j : j + 1],
                        lhsT=hfin[:, j * 128 : (j + 1) * 128].bitcast(f32),
                        rhs=consts_sb[:, 4:5],
                        start=True,
                        stop=True,
                    )
                e_sb = p3.tile([128, nj], f32)
                esum = p3.tile([128, 1], f32)
                nc.scalar.activation(e_sb, s_ps, AF.Exp, accum_out=esum)
                num_ps = p3ps.tile([128, 1], f32)
                for j in range(nj):
                    ht_ps = p3tp.tile([128, 128], f32, tag="htps")
                    nc.tensor.transpose(
                        ht_ps, hfin[:, j * 128 : (j + 1) * 128].bitcast(f32), ident_sb.bitcast(f32)
                    )
                    ht_sb = p3d.tile([128, 128], f32, tag="htsb")
                    nc.vector.tensor_copy(ht_sb, ht_ps)
                    nc.tensor.matmul(
                        num_ps,
                        lhsT=ht_sb,
                        rhs=e_sb[:, j : j + 1],
                        start=(j == 0),
                        stop=(j == nj - 1),
                        skip_group_check=True,
                    )
                out_sb = p3.tile([128, 2], f32)
                nc.vector.tensor_copy(out_sb[:, 0:1], num_ps)
                nc.vector.tensor_copy(out_sb[:, 1:2], esum)
                nc.sync.dma_start(out=partial[:, :], in_=out_sb)

    nc.compile()
    return nc


def _prep_inputs(H, E, W_ih, W_hh, b_ih, b_hh, attn_w):
    f = np.float32
    Hpad = np.concatenate(
        [np.zeros((VOCAB, WARM), np.float16), np.asarray(H).astype(np.float16)],
        axis=1,
    )
    Ec = np.ascontiguousarray(np.asarray(E).astype(np.float16))
    WihTc = np.ascontiguousarray(np.asarray(W_ih, dtype=f).T)
    WhhTc = np.ascontiguousarray(np.asarray(W_hh, dtype=f).T)
    ident = np.eye(128, dtype=f)
    b_ih = np.asarray(b_ih, dtype=f)
    b_hh = np.asarray(b_hh, dtype=f)
    in_maps = []
    for c in range(NCORES):
        consts = np.zeros((128, 8), f)
        consts[:, 0] = b_ih[0:128] + b_hh[0:128]
        consts[:, 1] = b_ih[128:256] + b_hh[128:256]
        consts[:, 2] = b_ih[256:384]
        consts[:, 3] = b_hh[256:384]
        consts[:, 4] = np.asarray(attn_w, dtype=f)
        consts[:, 5] = ZBOOST if c == 0 else 0.0
        in_maps.append(
            {
                "Hc": np.ascontiguousarray(Hpad[:, c * CHUNK : c * CHUNK + NCOL]),
                "Ein": Ec,
                "WihT": WihTc,
                "WhhT": WhhTc,
                "consts": consts,
                "ident": ident,
            }
        )
    return in_maps


def kernel(H, E, W_ih, W_hh, b_ih, b_hh, attn_w):
    from concourse.bass_utils import run_bass_kernel_spmd

    if "nc" not in _cache:
        _cache["nc"] = _build()
    nc = _cache["nc"]

    in_maps = _prep_inputs(H, E, W_ih, W_hh, b_ih, b_hh, attn_w)
    res = run_bass_kernel_spmd(nc, in_maps, core_ids=list(range(NCORES)))
    _cache["last_results"] = res

    num = np.zeros(128, np.float64)
    Z = 0.0
    for r in res.results:
        p = r["partial"]
        num += p[:, 0].astype(np.float64)
        Z += float(p[:, 1].sum(dtype=np.float64))
    return (num / Z).astype(np.float32)


# revision 41
# speedup vs baseline: 1.0273x; 1.0273x over previous
"""Trainium2 Bass kernel for nn_Encoder_20942260535802.

Pipeline (per reference.py):
    visit_emb = H.T @ E                      # [8192, 128]
    hidden    = GRU(visit_emb)               # sequential scan, 8192 steps
    out       = softmax(hidden@attn_w) @ hidden

Strategy:
  * Shard the 8192-visit sequence across 8 cores (1024 visits each). Each
    core loads only its H columns (memory-bound phase, ~20 MB/core) and
    computes visit_embT = E.T @ Hc plus the input-gate projections locally.
  * The GRU recurrence is solved by Jacobi (fixed-point) iteration over the
    whole local chunk: h_all <- GRUStep(shift(h_all), x). The GRU map is a
    contraction (L ~ 0.86/step empirically), so K sweeps give uniform error
    ~L^K; each sweep is batched GEMM + elementwise work with no per-step
    synchronization. A 64-step warmup halo (states start from h=0 at
    chunk_start-64) makes chunks fully independent: the boundary error
    decays as L^64 ~ 7e-5 (well under the sweep error), so no inter-core
    carry is needed at all.
  * Core 0 has no real predecessor: its halo's z-gate input gets +ZBOOST so
    z=1 and the state stays exactly 0 through the halo.
  * Sweeps are column-chunked (384/384/320) so consecutive sweeps pipeline
    as a wavefront across engines; PSUM holds r/z/n per chunk in 3 banks, with
    the gate inputs pre-seeded by identity matmuls so sigma reads a single
    accumulated PSUM region. Matmuls run in float32r (single-pass PE fp32,
    ~1.5e-4 rel err measured on HW); H/E are loaded as fp16 (halves the
    memory-bound phase; validated no accuracy impact). Phase 1 runs per
    column-chunk, spreads DMA issue across the SP/ACT/GPSIMD queues, and
    shares the PSUM pool with the sweeps so early sweeps overlap the
    remaining H DMA.
  * Attention pooling is computed per-core as partials (sum_t e_t*h_t [128]
    and per-partition partials of sum_t e_t); the final 8-way sum + divide
    is host-side unsharding (tiny).
  * Measured end-to-end vs float64 reference (K=38, f32r, fp16 loads,
    WARM=64): max rel err 6.6e-4. CoreSim cost-model exec estimate: ~214 us
    (ACT-bound: sigma+tanh sweep work at ~85% occupancy; per-sweep latency
    equals the ACT busy floor).
"""

import numpy as np

VOCAB, EDIM, HDIM = 4880, 128, 128
NSEQ = 8192
NCORES = 8
WARM = 64                  # warmup halo columns (err ~0.86^64 ~ 7e-5)
CHUNK = NSEQ // NCORES     # 1024
NCOL = CHUNK + WARM        # 1088 columns per core
KSWEEPS = 40               # Jacobi sweeps (err ~ 0.86^K)
ZBOOST = 60.0              # sigmoid(x+60) == 1.0 in fp32

_cache = {}


def _bank_chunks(off, length, bank=512):
    """Split [off, off+length) at `bank`-aligned boundaries (PSUM banks)."""
    res, c, end = [], off, off + length
    while c < end:
        w = min(bank - (c % bank), end - c)
        res.append((c, w))
        c += w
    return res


def _build():
    import concourse.mybir as mybir
    from concourse import bacc
    from concourse.tile import TileContext

    f32 = mybir.dt.float32
    f32r = mybir.dt.float32r  # single-pass PE fp32 (4x faster for N>=256)
    f16 = mybir.dt.float16    # halves the H/E DMA traffic; negligible error
    AF = mybir.ActivationFunctionType
    ALU = mybir.AluOpType

    nc = bacc.Bacc(None, target_bir_lowering=False, debug=False)

    Hc = nc.dram_tensor("Hc", [VOCAB, NCOL], f16, kind="ExternalInput")
    Ein = nc.dram_tensor("Ein", [VOCAB, EDIM], f16, kind="ExternalInput")
    WihT = nc.dram_tensor("WihT", [EDIM, 3 * HDIM], f32, kind="ExternalInput")
    WhhT = nc.dram_tensor("WhhT", [HDIM, 3 * HDIM], f32r, kind="ExternalInput")
    # consts cols: 0=b_r, 1=b_z (b_ih+b_hh), 2=b_ih_n, 3=b_hh_n, 4=attn_w, 5=zboost
    consts = nc.dram_tensor("consts", [128, 8], f32, kind="ExternalInput")
    ident = nc.dram_tensor("ident", [128, 128], f32r, kind="ExternalInput")
    partial = nc.dram_tensor("partial", [128, 2], f32, kind="ExternalOutput")

    KTILES = [(k * 128, min(128, VOCAB - k * 128)) for k in range((VOCAB + 127) // 128)]
    r_chunks = _bank_chunks(0, NCOL)            # r-gate span of ps_rz
    z_chunks = _bank_chunks(NCOL, NCOL)         # z-gate span of ps_rz
    n_chunks = _bank_chunks(0, NCOL)            # ps_n

    with TileContext(nc) as tc:
        with (
            tc.tile_pool(name="fixed", bufs=1) as fixed,
            tc.tile_pool(name="state", bufs=1) as state,
        ):
            consts_sb = fixed.tile([128, 8], f32)
            nc.sync.dma_start(out=consts_sb, in_=consts[:, :])
            ident_sb = fixed.tile([128, 128], f32r)
            nc.sync.dma_start(out=ident_sb, in_=ident[:, :])
            wih_f = fixed.tile([128, 3 * HDIM], f32)
            nc.sync.dma_start(out=wih_f, in_=WihT[:, :])
            wih_sb = fixed.tile([128, 3 * HDIM], f32r)
            nc.vector.tensor_copy(wih_sb, wih_f)
            whh_sb = fixed.tile([128, 3 * HDIM], f32r)
            nc.sync.dma_start(out=whh_sb, in_=WhhT[:, :])

            ve = state.tile([128, NCOL], f32r)           # visit_embT
            xpb_rz = state.tile([128, 2 * NCOL], f32r)   # x-proj + bias, r||z
            xnb = state.tile([128, NCOL], f32)          # x-proj + b_ih_n
            hA = state.tile([128, NCOL + 1], f32r)       # col 0 = left boundary (0)
            hB = state.tile([128, NCOL + 1], f32r)

            # ---------- Phases 1+2 interleaved ----------
            # Phase 1 runs per column-chunk (E.T @ Hc[:, chunk] -> gate
            # projections for that chunk), sharing one PSUM pool with the
            # sweeps so the first sweeps overlap the remaining H loads.
            nc.gpsimd.memset(hA.bitcast(f32), 0.0)
            nc.gpsimd.memset(hB.bitcast(f32), 0.0)
            CWS = [384, 384, 320]          # per-chunk widths (sum = NCOL)
            COFF = [0, 384, 768]
            NCH = len(CWS)
            assert sum(CWS) == NCOL
            NKT = len(KTILES)
            NKF = sum(1 for _, kp in KTILES if kp == 128)  # full 128-row tiles
            Eall = state.tile([128, NKT, EDIM], f16)
            nc.sync.dma_start(
                out=Eall[:, 0:NKF, :],
                in_=Ein[0 : NKF * 128, :].rearrange("(a p) d -> p a d", p=128),
            )
            nc.sync.dma_start(
                out=Eall[: VOCAB - NKF * 128, NKF, :], in_=Ein[NKF * 128 :, :]
            )
            GRP = 13
            kgroups = [KTILES[g : g + GRP] for g in range(0, NKT, GRP)]
            # spread H-load issue across engine queues: the SP sequencer's
            # per-dma_start cost (~2.4us) serializes phase 1 otherwise
            dma_engines = [nc.sync, nc.scalar, nc.gpsimd]
            dma_i = 0
            with (
                tc.tile_pool(name="p1ld", bufs=4) as p1ld,
                tc.tile_pool(name="sw", bufs=6) as sw,
                tc.tile_pool(name="pipeps", bufs=1, space="PSUM") as pps,
            ):
                for c in range(NCH):
                    cc0, CW = COFF[c], CWS[c]
                    ve_ps = pps.tile([128, 384], f32, tag="veps", name="ve_ps")[:, 0:CW]
                    for grp in kgroups:
                        ng = len(grp)
                        nfull = sum(1 for _, kp in grp if kp == 128)
                        k0 = grp[0][0]
                        h_t = p1ld.tile([128, GRP, CW], f16, tag="h")
                        eng = dma_engines[dma_i % len(dma_engines)]
                        dma_i += 1
                        if nfull:
                            eng.dma_start(
                                out=h_t[:, 0:nfull, :],
                                in_=Hc[k0 : k0 + 128 * nfull, cc0 : cc0 + CW].rearrange(
                                    "(a p) n -> p a n", p=128
                                ),
                            )
                        for a in range(nfull, ng):
                            ka, kp = grp[a]
                            eng.dma_start(
                                out=h_t[:kp, a, :], in_=Hc[ka : ka + kp, cc0 : cc0 + CW]
                            )
                        for a, (ka, kp) in enumerate(grp):
                            ki = ka // 128
                            nc.tensor.matmul(
                                ve_ps,
                                lhsT=Eall[:kp, ki, :],
                                rhs=h_t[:kp, a, :],
                                start=(ki == 0),
                                stop=(ki == NKT - 1),
                            )
                    nc.vector.tensor_copy(ve[:, cc0 : cc0 + CW], ve_ps)
                    for g in range(3):
                        gp = pps.tile([128, CW], f32, tag="gp")
                        nc.tensor.matmul(
                            gp,
                            lhsT=wih_sb[:, g * 128 : (g + 1) * 128],
                            rhs=ve[:, cc0 : cc0 + CW],
                            start=True,
                            stop=True,
                        )
                        if g == 0:
                            nc.vector.tensor_scalar_add(
                                xpb_rz[:, cc0 : cc0 + CW], gp, consts_sb[:, 0:1]
                            )
                        elif g == 1:
                            if c == 0:
                                # warmup cols: +zboost (core 0 only; 0 elsewhere)
                                nc.vector.tensor_scalar(
                                    xpb_rz[:, NCOL : NCOL + WARM],
                                    gp[:, 0:WARM],
                                    consts_sb[:, 1:2],
                                    consts_sb[:, 5:6],
                                    ALU.add,
                                    ALU.add,
                                )
                                nc.vector.tensor_scalar_add(
                                    xpb_rz[:, NCOL + WARM : NCOL + CW],
                                    gp[:, WARM:],
                                    consts_sb[:, 1:2],
                                )
                            else:
                                nc.vector.tensor_scalar_add(
                                    xpb_rz[:, NCOL + cc0 : NCOL + cc0 + CW],
                                    gp,
                                    consts_sb[:, 1:2],
                                )
                        else:
                            nc.vector.tensor_scalar_add(
                                xnb[:, cc0 : cc0 + CW], gp, consts_sb[:, 2:3]
                            )

                # ---------- Phase 2: Jacobi sweeps (wavefront) ----------
                for k in range(KSWEEPS):
                    hprev_t, hnext_t = (hA, hB) if k % 2 == 0 else (hB, hA)
                    for c in range(NCH):
                        c0, CW = COFF[c], CWS[c]
                        hp = hprev_t[:, c0 : c0 + CW]
                        # psum layout: r at [0:CW] (bank 0), z at [512:512+CW]
                        # (bank 1), n at [1024:1024+CW] (bank 2)
                        ps_rz = pps.tile(
                            [128, 1024], f32, tag="psrz", bufs=2, name="ps_rz"
                        )
                        ps_n = pps.tile(
                            [128, 384], f32, tag="psn", bufs=2, name="ps_n"
                        )
                        nc.tensor.matmul(
                            ps_rz[:, 0:CW],
                            lhsT=ident_sb,
                            rhs=xpb_rz[:, c0 : c0 + CW],
                            start=True,
                            stop=False,
                        )
                        nc.tensor.matmul(
                            ps_rz[:, 512 : 512 + CW],
                            lhsT=ident_sb,
                            rhs=xpb_rz[:, NCOL + c0 : NCOL + c0 + CW],
                            start=True,
                            stop=False,
                        )
                        nc.tensor.matmul(
                            ps_rz[:, 0:CW],
                            lhsT=whh_sb[:, 0:128],
                            rhs=hp,
                            start=False,
                            stop=True,
                        )
                        nc.tensor.matmul(
                            ps_rz[:, 512 : 512 + CW],
                            lhsT=whh_sb[:, 128:256],
                            rhs=hp,
                            start=False,
                            stop=True,
                        )
                        nc.tensor.matmul(
                            ps_n[:, 0:CW],
                            lhsT=whh_sb[:, 256:384],
                            rhs=hp,
                            start=True,
                            stop=True,
                        )
                        # sigmoid over r and z via a strided view (skips pads)
                        rz = sw.tile([128, 2, CW], f32, tag="rz")
                        ps_rz_view = ps_rz.rearrange("p (b y) -> p b y", y=512)[
                            :, 0:2, 0:CW
                        ]
                        nc.scalar.activation(rz, ps_rz_view, AF.Sigmoid)
                        r = rz[:, 0, :]
                        z = rz[:, 1, :]
                        m = sw.tile([128, CW], f32, tag="m")
                        # m = (ghn + b_hh_n) * r
                        nc.vector.scalar_tensor_tensor(
                            m,
                            ps_n[:, 0:CW],
                            consts_sb[:, 3:4],
                            r,
                            ALU.add,
                            ALU.mult,
                        )
                        t3 = sw.tile([128, CW], f32, tag="t3")
                        nc.gpsimd.tensor_add(t3, m, xnb[:, c0 : c0 + CW])
                        nt = sw.tile([128, CW], f32, tag="nt")
                        nc.scalar.activation(nt, t3, AF.Tanh)
                        # h' = (1-z)*n + z*hprev; zb and u run parallel to
                        # the tanh path so only 2 ops trail the tanh
                        zb = sw.tile([128, CW], f32, tag="zb")
                        nc.gpsimd.tensor_scalar(
                            zb, z, -1.0, 1.0, ALU.mult, ALU.add
                        )
                        u = sw.tile([128, CW], f32, tag="u")
                        nc.vector.tensor_mul(u, z, hp.bitcast(f32))
                        m3 = sw.tile([128, CW], f32, tag="m3")
                        nc.gpsimd.tensor_mul(m3, nt, zb)
                        nc.gpsimd.tensor_add(
                            hnext_t[:, 1 + c0 : 1 + c0 + CW], m3, u
                        )

            # ---------- Phase 3: attention pooling (partial) ----------
            hfin = (hA if KSWEEPS % 2 == 0 else hB)[:, 1 + WARM : 1 + NCOL]
            with (
                tc.tile_pool(name="p3", bufs=1) as p3,
                tc.tile_pool(name="p3d", bufs=2) as p3d,
                tc.tile_pool(name="p3ps", bufs=1, space="PSUM") as p3ps,
                tc.tile_pool(name="p3tp", bufs=2, space="PSUM") as p3tp,
            ):
                nj = CHUNK // 128
                s_ps = p3ps.tile([128, nj], f32)
                for j in range(nj):
                    nc.tensor.matmul(
                        s_ps[:, j : j + 1],
                        lhsT=hfin[:, j * 128 : (j + 1) * 128].bitcast(f32),
                        rhs=consts_sb[:, 4:5],
                        start=True,
                        stop=True,
                    )
                e_sb = p3.tile([128, nj], f32)
                esum = p3.tile([128, 1], f32)
                nc.scalar.activation(e_sb, s_ps, AF.Exp, accum_out=esum)
                num_ps = p3ps.tile([128, 1], f32)
                for j in range(nj):
                    ht_ps = p3tp.tile([128, 128], f32, tag="htps")
                    nc.tensor.transpose(
                        ht_ps, hfin[:, j * 128 : (j + 1) * 128].bitcast(f32), ident_sb.bitcast(f32)
                    )
                    ht_sb = p3d.tile([128, 128], f32, tag="htsb")
                    nc.vector.tensor_copy(ht_sb, ht_ps)
                    nc.tensor.matmul(
                        num_ps,
                        lhsT=ht_sb,
                        rhs=e_sb[:, j : j + 1],
                        start=(j == 0),
                        stop=(j == nj - 1),
                        skip_group_check=True,
                    )
                out_sb = p3.tile([128, 2], f32)
                nc.vector.tensor_copy(out_sb[:, 0:1], num_ps)
                nc.vector.tensor_copy(out_sb[:, 1:2], esum)
                nc.sync.dma_start(out=partial[:, :], in_=out_sb)

    nc.compile()
    return nc


def _prep_inputs(H, E, W_ih, W_hh, b_ih, b_hh, attn_w):
    f = np.float32
    Hpad = np.concatenate(
        [np.zeros((VOCAB, WARM), np.float16), np.asarray(H).astype(np.float16)],
        axis=1,
    )
    Ec = np.ascontiguousarray(np.asarray(E).astype(np.float16))
    WihTc = np.ascontiguousarray(np.asarray(W_ih, dtype=f).T)
    WhhTc = np.ascontiguousarray(np.asarray(W_hh, dtype=f).T)
    ident = np.eye(128, dtype=f)
    b_ih = np.asarray(b_ih, dtype=f)
    b_hh = np.asarray(b_hh, dtype=f)
    in_maps = []
    for c in range(NCORES):
        consts = np.zeros((128, 8), f)
        consts[:, 0] = b_ih[0:128] + b_hh[0:128]
        consts[:, 1] = b_ih[128:256] + b_hh[128:256]
        consts[:, 2] = b_ih[256:384]
        consts[:, 3] = b_hh[256:384]
        consts[:, 4] = np.asarray(attn_w, dtype=f)
        consts[:, 5] = ZBOOST if c == 0 else 0.0
        in_maps.append(
            {
                "Hc": np.ascontiguousarray(Hpad[:, c * CHUNK : c * CHUNK + NCOL]),
                "Ein": Ec,
                "WihT": WihTc,
                "WhhT": WhhTc,
                "consts": consts,
                "ident": ident,
            }
        )
    return in_maps


def kernel(H, E, W_ih, W_hh, b_ih, b_hh, attn_w):
    from concourse.bass_utils import run_bass_kernel_spmd

    if "nc" not in _cache:
        _cache["nc"] = _build()
    nc = _cache["nc"]

    in_maps = _prep_inputs(H, E, W_ih, W_hh, b_ih, b_hh, attn_w)
    res = run_bass_kernel_spmd(nc, in_maps, core_ids=list(range(NCORES)))
    _cache["last_results"] = res

    num = np.zeros(128, np.float64)
    Z = 0.0
    for r in res.results:
        p = r["partial"]
        num += p[:, 0].astype(np.float64)
        Z += float(p[:, 1].sum(dtype=np.float64))
    return (num / Z).astype(np.float32)


# revision 44
# speedup vs baseline: 1.0488x; 1.0209x over previous
"""Trainium2 Bass kernel for nn_Encoder_20942260535802.

Pipeline (per reference.py):
    visit_emb = H.T @ E                      # [8192, 128]
    hidden    = GRU(visit_emb)               # sequential scan, 8192 steps
    out       = softmax(hidden@attn_w) @ hidden

Strategy:
  * Shard the 8192-visit sequence across 8 cores (1024 visits each). Each
    core loads only its H columns (memory-bound phase, ~20 MB/core) and
    computes visit_embT = E.T @ Hc plus the input-gate projections locally.
  * The GRU recurrence is solved by Jacobi (fixed-point) iteration over the
    whole local chunk: h_all <- GRUStep(shift(h_all), x). The GRU map is a
    contraction (L ~ 0.86/step empirically), so K sweeps give uniform error
    ~L^K; each sweep is batched GEMM + elementwise work with no per-step
    synchronization. A 64-step warmup halo (states start from h=0 at
    chunk_start-64) makes chunks fully independent: the boundary error
    decays as L^64 ~ 7e-5 (well under the sweep error), so no inter-core
    carry is needed at all.
  * Core 0 has no real predecessor: its halo's z-gate input gets +ZBOOST so
    z=1 and the state stays exactly 0 through the halo.
  * Sweeps are column-chunked (384/384/320) so consecutive sweeps pipeline
    as a wavefront across engines; PSUM holds r/z (2 banks) and n (1 bank)
    in independently double-buffered tiles per chunk, with
    the gate inputs pre-seeded by identity matmuls so sigma reads a single
    accumulated PSUM region. Matmuls run in float32r (single-pass PE fp32,
    ~1.5e-4 rel err measured on HW); H/E are loaded as fp16 (halves the
    memory-bound phase; validated no accuracy impact). Phase 1 runs per
    column-chunk, spreads DMA issue across the SP/ACT/GPSIMD queues, and
    shares the PSUM pool with the sweeps so early sweeps overlap the
    remaining H DMA.
  * Attention pooling is computed per-core as partials (sum_t e_t*h_t [128]
    and per-partition partials of sum_t e_t); the final 8-way sum + divide
    is host-side unsharding (tiny).
  * Measured end-to-end vs float64 reference (K=38, f32r, fp16 loads,
    WARM=64): max rel err 6.6e-4. CoreSim cost-model exec estimate: ~208 us
    (ACT-bound: sigma+tanh sweep work at ~85% occupancy; per-sweep latency
    equals the ACT busy floor).
"""

import numpy as np

VOCAB, EDIM, HDIM = 4880, 128, 128
NSEQ = 8192
NCORES = 8
WARM = 64                  # warmup halo columns (err ~0.86^64 ~ 7e-5)
CHUNK = NSEQ // NCORES     # 1024
NCOL = CHUNK + WARM        # 1088 columns per core
KSWEEPS = 40               # Jacobi sweeps (err ~ 0.86^K)
ZBOOST = 60.0              # sigmoid(x+60) == 1.0 in fp32

_cache = {}


def _bank_chunks(off, length, bank=512):
    """Split [off, off+length) at `bank`-aligned boundaries (PSUM banks)."""
    res, c, end = [], off, off + length
    while c < end:
        w = min(bank - (c % bank), end - c)
        res.append((c, w))
        c += w
    return res


def _build():
    import concourse.mybir as mybir
    from concourse import bacc
    from concourse.tile import TileContext

    f32 = mybir.dt.float32
    f32r = mybir.dt.float32r  # single-pass PE fp32 (4x faster for N>=256)
    f16 = mybir.dt.float16    # halves the H/E DMA traffic; negligible error
    AF = mybir.ActivationFunctionType
    ALU = mybir.AluOpType

    nc = bacc.Bacc(None, target_bir_lowering=False, debug=False)

    Hc = nc.dram_tensor("Hc", [VOCAB, NCOL], f16, kind="ExternalInput")
    Ein = nc.dram_tensor("Ein", [VOCAB, EDIM], f16, kind="ExternalInput")
    WihT = nc.dram_tensor("WihT", [EDIM, 3 * HDIM], f32, kind="ExternalInput")
    WhhT = nc.dram_tensor("WhhT", [HDIM, 3 * HDIM], f32r, kind="ExternalInput")
    # consts cols: 0=b_r, 1=b_z (b_ih+b_hh), 2=b_ih_n, 3=b_hh_n, 4=attn_w, 5=zboost
    consts = nc.dram_tensor("consts", [128, 8], f32, kind="ExternalInput")
    ident = nc.dram_tensor("ident", [128, 128], f32r, kind="ExternalInput")
    partial = nc.dram_tensor("partial", [128, 2], f32, kind="ExternalOutput")

    KTILES = [(k * 128, min(128, VOCAB - k * 128)) for k in range((VOCAB + 127) // 128)]
    r_chunks = _bank_chunks(0, NCOL)            # r-gate span of ps_rz
    z_chunks = _bank_chunks(NCOL, NCOL)         # z-gate span of ps_rz
    n_chunks = _bank_chunks(0, NCOL)            # ps_n

    with TileContext(nc) as tc:
        with (
            tc.tile_pool(name="fixed", bufs=1) as fixed,
            tc.tile_pool(name="state", bufs=1) as state,
        ):
            consts_sb = fixed.tile([128, 8], f32)
            nc.sync.dma_start(out=consts_sb, in_=consts[:, :])
            ident_sb = fixed.tile([128, 128], f32r)
            nc.sync.dma_start(out=ident_sb, in_=ident[:, :])
            wih_f = fixed.tile([128, 3 * HDIM], f32)
            nc.sync.dma_start(out=wih_f, in_=WihT[:, :])
            wih_sb = fixed.tile([128, 3 * HDIM], f32r)
            nc.vector.tensor_copy(wih_sb, wih_f)
            whh_sb = fixed.tile([128, 3 * HDIM], f32r)
            nc.sync.dma_start(out=whh_sb, in_=WhhT[:, :])

            ve = state.tile([128, NCOL], f32r)           # visit_embT
            xpb_rz = state.tile([128, 2 * NCOL], f32r)   # x-proj + bias, r||z
            xnb = state.tile([128, NCOL], f32)          # x-proj + b_ih_n
            hA = state.tile([128, NCOL + 1], f32r)       # col 0 = left boundary (0)
            hB = state.tile([128, NCOL + 1], f32r)

            # ---------- Phases 1+2 interleaved ----------
            # Phase 1 runs per column-chunk (E.T @ Hc[:, chunk] -> gate
            # projections for that chunk), sharing one PSUM pool with the
            # sweeps so the first sweeps overlap the remaining H loads.
            nc.gpsimd.memset(hA.bitcast(f32), 0.0)
            nc.gpsimd.memset(hB.bitcast(f32), 0.0)
            CWS = [384, 384, 320]          # per-chunk widths (sum = NCOL)
            COFF = [0, 384, 768]
            NCH = len(CWS)
            assert sum(CWS) == NCOL
            NKT = len(KTILES)
            NKF = sum(1 for _, kp in KTILES if kp == 128)  # full 128-row tiles
            Eall = state.tile([128, NKT, EDIM], f16)
            nc.sync.dma_start(
                out=Eall[:, 0:NKF, :],
                in_=Ein[0 : NKF * 128, :].rearrange("(a p) d -> p a d", p=128),
            )
            nc.sync.dma_start(
                out=Eall[: VOCAB - NKF * 128, NKF, :], in_=Ein[NKF * 128 :, :]
            )
            GRP = 13
            kgroups = [KTILES[g : g + GRP] for g in range(0, NKT, GRP)]
            # spread H-load issue across engine queues: the SP sequencer's
            # per-dma_start cost (~2.4us) serializes phase 1 otherwise
            dma_engines = [nc.sync, nc.scalar, nc.gpsimd]
            dma_i = 0
            with (
                tc.tile_pool(name="p1ld", bufs=4) as p1ld,
                tc.tile_pool(name="sw", bufs=6) as sw,
                tc.tile_pool(name="pipeps", bufs=1, space="PSUM") as pps,
            ):
                for c in range(NCH):
                    cc0, CW = COFF[c], CWS[c]
                    ve_ps = pps.tile([128, 384], f32, tag="veps", name="ve_ps")[:, 0:CW]
                    for grp in kgroups:
                        ng = len(grp)
                        nfull = sum(1 for _, kp in grp if kp == 128)
                        k0 = grp[0][0]
                        h_t = p1ld.tile([128, GRP, CW], f16, tag="h")
                        eng = dma_engines[dma_i % len(dma_engines)]
                        dma_i += 1
                        if nfull:
                            eng.dma_start(
                                out=h_t[:, 0:nfull, :],
                                in_=Hc[k0 : k0 + 128 * nfull, cc0 : cc0 + CW].rearrange(
                                    "(a p) n -> p a n", p=128
                                ),
                            )
                        for a in range(nfull, ng):
                            ka, kp = grp[a]
                            eng.dma_start(
                                out=h_t[:kp, a, :], in_=Hc[ka : ka + kp, cc0 : cc0 + CW]
                            )
                        for a, (ka, kp) in enumerate(grp):
                            ki = ka // 128
                            nc.tensor.matmul(
                                ve_ps,
                                lhsT=Eall[:kp, ki, :],
                                rhs=h_t[:kp, a, :],
                                start=(ki == 0),
                                stop=(ki == NKT - 1),
                            )
                    nc.vector.tensor_copy(ve[:, cc0 : cc0 + CW], ve_ps)
                    for g in range(3):
                        gp = pps.tile([128, CW], f32, tag="gp")
                        nc.tensor.matmul(
                            gp,
                            lhsT=wih_sb[:, g * 128 : (g + 1) * 128],
                            rhs=ve[:, cc0 : cc0 + CW],
                            start=True,
                            stop=True,
                        )
                        if g == 0:
                            nc.vector.tensor_scalar_add(
                                xpb_rz[:, cc0 : cc0 + CW], gp, consts_sb[:, 0:1]
                            )
                        elif g == 1:
                            if c == 0:
                                # warmup cols: +zboost (core 0 only; 0 elsewhere)
                                nc.vector.tensor_scalar(
                                    xpb_rz[:, NCOL : NCOL + WARM],
                                    gp[:, 0:WARM],
                                    consts_sb[:, 1:2],
                                    consts_sb[:, 5:6],
                                    ALU.add,
                                    ALU.add,
                                )
                                nc.vector.tensor_scalar_add(
                                    xpb_rz[:, NCOL + WARM : NCOL + CW],
                                    gp[:, WARM:],
                                    consts_sb[:, 1:2],
                                )
                            else:
                                nc.vector.tensor_scalar_add(
                                    xpb_rz[:, NCOL + cc0 : NCOL + cc0 + CW],
                                    gp,
                                    consts_sb[:, 1:2],
                                )
                        else:
                            nc.vector.tensor_scalar_add(
                                xnb[:, cc0 : cc0 + CW], gp, consts_sb[:, 2:3]
                            )

                # ---------- Phase 2: Jacobi sweeps (wavefront) ----------
                for k in range(KSWEEPS):
                    hprev_t, hnext_t = (hA, hB) if k % 2 == 0 else (hB, hA)
                    for c in range(NCH):
                        c0, CW = COFF[c], CWS[c]
                        hp = hprev_t[:, c0 : c0 + CW]
                        # psum layout: r at [0:CW] (bank 0), z at [512:512+CW]
                        # (bank 1), n at [1024:1024+CW] (bank 2)
                        ps_rz = pps.tile(
                            [128, 1024], f32, tag="psrz", bufs=2, name="ps_rz"
                        )
                        ps_n = pps.tile(
                            [128, 384], f32, tag="psn", bufs=2, name="ps_n"
                        )
                        nc.tensor.matmul(
                            ps_rz[:, 0:CW],
                            lhsT=ident_sb,
                            rhs=xpb_rz[:, c0 : c0 + CW],
                            start=True,
                            stop=False,
                        )
                        nc.tensor.matmul(
                            ps_rz[:, 512 : 512 + CW],
                            lhsT=ident_sb,
                            rhs=xpb_rz[:, NCOL + c0 : NCOL + c0 + CW],
                            start=True,
                            stop=False,
                        )
                        nc.tensor.matmul(
                            ps_rz[:, 0:CW],
                            lhsT=whh_sb[:, 0:128],
                            rhs=hp,
                            start=False,
                            stop=True,
                        )
                        nc.tensor.matmul(
                            ps_rz[:, 512 : 512 + CW],
                            lhsT=whh_sb[:, 128:256],
                            rhs=hp,
                            start=False,
                            stop=True,
                        )
                        nc.tensor.matmul(
                            ps_n[:, 0:CW],
                            lhsT=whh_sb[:, 256:384],
                            rhs=hp,
                            start=True,
                            stop=True,
                        )
                        # sigmoid over r and z via a strided view (skips pads)
                        rz = sw.tile([128, 2, CW], f32, tag="rz")
                        ps_rz_view = ps_rz.rearrange("p (b y) -> p b y", y=512)[
                            :, 0:2, 0:CW
                        ]
                        nc.scalar.activation(rz, ps_rz_view, AF.Sigmoid)
                        r = rz[:, 0, :]
                        z = rz[:, 1, :]
                        m = sw.tile([128, CW], f32, tag="m")
                        # m = (ghn + b_hh_n) * r
                        nc.vector.scalar_tensor_tensor(
                            m,
                            ps_n[:, 0:CW],
                            consts_sb[:, 3:4],
                            r,
                            ALU.add,
                            ALU.mult,
                        )
                        t3 = sw.tile([128, CW], f32, tag="t3")
                        nc.gpsimd.tensor_add(t3, m, xnb[:, c0 : c0 + CW])
                        nt = sw.tile([128, CW], f32, tag="nt")
                        nc.scalar.activation(nt, t3, AF.Tanh)
                        # h' = (1-z)*n + z*hprev; zb and u run parallel to
                        # the tanh path so only 2 ops trail the tanh
                        zb = sw.tile([128, CW], f32, tag="zb")
                        nc.gpsimd.tensor_scalar(
                            zb, z, -1.0, 1.0, ALU.mult, ALU.add
                        )
                        u = sw.tile([128, CW], f32, tag="u")
                        nc.vector.tensor_mul(u, z, hp.bitcast(f32))
                        m3 = sw.tile([128, CW], f32, tag="m3")
                        nc.gpsimd.tensor_mul(m3, nt, zb)
                        nc.gpsimd.tensor_add(
                            hnext_t[:, 1 + c0 : 1 + c0 + CW], m3, u
                        )

            # ---------- Phase 3: attention pooling (partial) ----------
            hfin = (hA if KSWEEPS % 2 == 0 else hB)[:, 1 + WARM : 1 + NCOL]
            with (
                tc.tile_pool(name="p3", bufs=1) as p3,
                tc.tile_pool(name="p3d", bufs=2) as p3d,
                tc.tile_pool(name="p3ps", bufs=1, space="PSUM") as p3ps,
                tc.tile_pool(name="p3tp", bufs=2, space="PSUM") as p3tp,
            ):
                nj = CHUNK // 128
                s_ps = p3ps.tile([128, nj], f32)
                for j in range(nj):
                    nc.tensor.matmul(
                        s_ps[:, j : j + 1],
                        lhsT=hfin[:, j * 128 : (j + 1) * 128].bitcast(f32),
                        rhs=consts_sb[:, 4:5],
                        start=True,
                        stop=True,
                    )
                e_sb = p3.tile([128, nj], f32)
                esum = p3.tile([128, 1], f32)
                nc.scalar.activation(e_sb, s_ps, AF.Exp, accum_out=esum)
                num_ps = p3ps.tile([128, 1], f32)
                for j in range(nj):
                    ht_ps = p3tp.tile([128, 128], f32, tag="htps")
                    nc.tensor.transpose(
                        ht_ps, hfin[:, j * 128 : (j + 1) * 128].bitcast(f32), ident_sb.bitcast(f32)
                    )
                    ht_sb = p3d.tile([128, 128], f32, tag="htsb")
                    nc.vector.tensor_copy(ht_sb, ht_ps)
                    nc.tensor.matmul(
                        num_ps,
                        lhsT=ht_sb,
                        rhs=e_sb[:, j : j + 1],
                        start=(j == 0),
                        stop=(j == nj - 1),
                        skip_group_check=True,
                    )
                out_sb = p3.tile([128, 2], f32)
                nc.vector.tensor_copy(out_sb[:, 0:1], num_ps)
                nc.vector.tensor_copy(out_sb[:, 1:2], esum)
                nc.sync.dma_start(out=partial[:, :], in_=out_sb)

    nc.compile()
    return nc


def _prep_inputs(H, E, W_ih, W_hh, b_ih, b_hh, attn_w):
    f = np.float32
    Hpad = np.concatenate(
        [np.zeros((VOCAB, WARM), np.float16), np.asarray(H).astype(np.float16)],
        axis=1,
    )
    Ec = np.ascontiguousarray(np.asarray(E).astype(np.float16))
    WihTc = np.ascontiguousarray(np.asarray(W_ih, dtype=f).T)
    WhhTc = np.ascontiguousarray(np.asarray(W_hh, dtype=f).T)
    ident = np.eye(128, dtype=f)
    b_ih = np.asarray(b_ih, dtype=f)
    b_hh = np.asarray(b_hh, dtype=f)
    in_maps = []
    for c in range(NCORES):
        consts = np.zeros((128, 8), f)
        consts[:, 0] = b_ih[0:128] + b_hh[0:128]
        consts[:, 1] = b_ih[128:256] + b_hh[128:256]
        consts[:, 2] = b_ih[256:384]
        consts[:, 3] = b_hh[256:384]
        consts[:, 4] = np.asarray(attn_w, dtype=f)
        consts[:, 5] = ZBOOST if c == 0 else 0.0
        in_maps.append(
            {
                "Hc": np.ascontiguousarray(Hpad[:, c * CHUNK : c * CHUNK + NCOL]),
                "Ein": Ec,
                "WihT": WihTc,
                "WhhT": WhhTc,
                "consts": consts,
                "ident": ident,
            }
        )
    return in_maps


def kernel(H, E, W_ih, W_hh, b_ih, b_hh, attn_w):
    from concourse.bass_utils import run_bass_kernel_spmd

    if "nc" not in _cache:
        _cache["nc"] = _build()
    nc = _cache["nc"]

    in_maps = _prep_inputs(H, E, W_ih, W_hh, b_ih, b_hh, attn_w)
    res = run_bass_kernel_spmd(nc, in_maps, core_ids=list(range(NCORES)))
    _cache["last_results"] = res

    num = np.zeros(128, np.float64)
    Z = 0.0
    for r in res.results:
        p = r["partial"]
        num += p[:, 0].astype(np.float64)
        Z += float(p[:, 1].sum(dtype=np.float64))
    return (num / Z).astype(np.float32)
